# revision 50
# baseline (speedup 1.0000x reference)
"""Trainium2 Bass kernel for nn_AttentionBlock_51685636440106.

Math: with S=2048, num_heads=1024 -> 2 tokens per head, the block-diagonal
attention over permuted tokens reduces to a pairwise sigmoid blend:
  for each adjacent pair (in permuted order) t, p=t^1:
    w_t  = sigmoid((Q_t.(K_t - K_p)) / sqrt(H))
    att_t = w_t * V_t + (1-w_t) * V_p
Softmax over two logits depends only on their difference and the blend is
linear in V, so two weight products fold on the host:
  Wqk = Wq @ Wk.T -> dd_t = (xp @ Wqk)_t . (xp_t - xp_p) / 32
  Wvo = Wv @ Wo   -> VO = xp @ Wvo;  y_t = w_t*VO_t + (1-w_t)*VO_p
The permutation is applied on the host (gather before, scatter after); the
device kernel sees contiguous permuted tokens.

Sharding: token-parallel over B*S = 8192 tokens across 8 cores (1024 each;
pairs are adjacent and never cross the even-aligned shard boundaries).

All matmuls run as fp32r (full-rate fp32 PE mode, ~1.5e-4 relative
accuracy). Trivial vector params (zero biases / unit gains), detected on
the host, compile out of the kernel; a general fallback path remains.
"""

import numpy as np

import concourse.bass as bass
import concourse.mybir as mybir
import concourse.tile as tile
from concourse import bacc
from concourse.bass_utils import run_bass_kernel_spmd
from concourse.masks import make_identity

F32 = mybir.dt.float32
F32R = mybir.dt.float32r
AF = mybir.ActivationFunctionType
OP = mybir.AluOpType

N_CORES = 8
B, S, H = 4, 2048, 1024
T = B * S // N_CORES        # tokens per core
TT = T // 128               # token tiles per core (8)
KT = H // 128               # contraction tiles over H (8)
F1 = 4 * H                  # FFN hidden (4096)
MT = F1 // 128              # FFN feature tiles (32)
LN_EPS = 1e-5
SCALE = 1.0 / 32.0          # 1/sqrt(H)
SWAP_MASK = [i ^ 1 for i in range(32)]


def build_nc(triv=(True, True, True, True, True)):
    """triv = (ck==0, cvo==0, g1==1 and be1==0, b2==0, g2==1 and be2==0)"""
    t_ck, t_cvo, t_ln1, t_b2, t_ln2 = triv
    nc = bacc.Bacc(None, target_bir_lowering=False)

    xp_d = nc.dram_tensor("xp", [TT, 128, H], F32, kind="ExternalInput")
    wqk_d = nc.dram_tensor("wqk", [KT, 128, H], F32R, kind="ExternalInput")
    wvo_d = nc.dram_tensor("wvo", [KT, 128, H], F32R, kind="ExternalInput")
    w1_d = nc.dram_tensor("w1", [KT, 128, F1], F32R, kind="ExternalInput")
    w2_d = nc.dram_tensor("w2", [MT, 128, H], F32R, kind="ExternalInput")
    b1_d = nc.dram_tensor("b1t", [128, MT], F32, kind="ExternalInput")
    # rows: 0 ck (Wk@bq), 1 cvo (bv@Wo + bo), 2 g1, 3 be1, 4 g2, 5 be2, 6 b2
    vec_d = nc.dram_tensor("vecs", [7, H], F32, kind="ExternalInput")
    out_d = nc.dram_tensor("out", [TT, 128, H], F32, kind="ExternalOutput")

    def bcast_row(i):
        v = vec_d[i, :]
        return bass.AP(tensor=v.tensor, offset=v.offset, ap=[[0, 128], [1, H]])

    all_triv = t_ck and t_cvo and t_ln1 and t_b2 and t_ln2
    dxt_bufs = 2
    wp_bufs = 3 if all_triv else 2

    with tile.TileContext(nc) as tc:
        with (
            tc.tile_pool(name="big", bufs=1) as big,
            tc.tile_pool(name="wp", bufs=wp_bufs) as wp,
            tc.tile_pool(name="w1p", bufs=4) as w1p,
            tc.tile_pool(name="vbc", bufs=2) as vbc,
            tc.tile_pool(name="shufp", bufs=1) as shufp,
            tc.tile_pool(name="dxp", bufs=dxt_bufs) as dxp,
            tc.tile_pool(name="stat", bufs=4) as stat,
            tc.tile_pool(name="cst", bufs=1) as cst,
            tc.tile_pool(name="psmm", bufs=4, space="PSUM") as psmm,
            tc.tile_pool(name="psmm2", bufs=2, space="PSUM") as psmm2,
            tc.tile_pool(name="pstp", bufs=2, space="PSUM") as pstp,
        ):
            ident = cst.tile([128, 128], F32, tag="ident")
            make_identity(nc, ident)
            epst = cst.tile([128, 1], F32, tag="eps")
            nc.gpsimd.memset(epst[:], LN_EPS)
            b1t = cst.tile([128, MT], F32, tag="b1t")
            nc.sync.dma_start(out=b1t[:], in_=b1_d[:, :])

            # ---- load xp (token-major); wqk interleaved so G isn't DMA-starved ----
            xp = big.tile([128, TT, H], F32, tag="bufX")
            for q in range(4):
                nc.sync.dma_start(out=xp[:, 0, q * 256:(q + 1) * 256],
                                  in_=xp_d[0, :, q * 256:(q + 1) * 256])
            wqk_sb = []
            for nh in range(2):
                wt = wp.tile([128, KT, 512], F32R, tag="wset")
                for k in range(KT):
                    nc.sync.dma_start(out=wt[:, k, :],
                                      in_=wqk_d[k, :, nh * 512:(nh + 1) * 512])
                wqk_sb.append(wt)
            for t in range(1, TT):
                nc.sync.dma_start(out=xp[:, t, :], in_=xp_d[t, :, :])

            # ---- xpT = transposed xp (feature-major, fp32r) ----
            xpT = big.tile([128, KT, TT, 128], F32R, tag="bufT")
            for t in range(TT):
                for k in range(KT):
                    pst = pstp.tile([128, 128], F32, tag="tp")
                    nc.tensor.transpose(pst[:], xp[:, t, k * 128:(k + 1) * 128], ident[:])
                    nc.scalar.copy(out=xpT[:, k, t, :], in_=pst[:])

            # ---- dx_t = xp_t - xp_pair(t) for every tile (overlaps G matmuls) ----
            dxs = []
            for t in range(TT):
                xs = shufp.tile([128, H], F32, tag="shuf")
                nc.vector.stream_shuffle(xs[:], xp[:, t, :], SWAP_MASK)
                dxt = dxp.tile([128, H], F32, tag="dxt")
                nc.gpsimd.tensor_sub(dxt[:], xp[:, t, :], xs[:])
                dxs.append(dxt)

            ck_b = None
            if not t_ck:
                ck_b = vbc.tile([128, H], F32, tag="vbc")
                nc.sync.dma_start(out=ck_b[:], in_=bcast_row(0))

            # ---- G psums consumed in place: dd_t = (G_t . dx_t), w = sigmoid ----
            sigw = cst.tile([128, TT], F32, tag="sigw")
            sigw1m = cst.tile([128, TT], F32, tag="sigw1m")
            for t in range(TT):
                dxt = dxs[t]
                ddt = stat.tile([128, 1], F32, tag="dd")
                for nh in range(2):
                    sl = slice(nh * 512, (nh + 1) * 512)
                    ps = psmm.tile([128, 512], F32, tag="mm")
                    for k in range(KT):
                        nc.tensor.matmul(ps[:], xpT[:, k, t, :], wqk_sb[nh][:, k, :],
                                         start=(k == 0), stop=(k == KT - 1))
                    if t_ck:
                        nc.vector.tensor_mul(dxt[:, sl], ps[:], dxt[:, sl])
                    else:
                        nc.vector.tensor_add(ps[:], ps[:], ck_b[:, sl])
                        nc.vector.tensor_mul(dxt[:, sl], ps[:], dxt[:, sl])
                nc.vector.reduce_sum(ddt[:], dxt[:], axis=mybir.AxisListType.X)
                nc.scalar.activation(sigw[:, t:t + 1], ddt[:], AF.Sigmoid, scale=SCALE)
                nc.scalar.activation(sigw1m[:, t:t + 1], ddt[:], AF.Sigmoid, scale=-SCALE)

            # ---- VO = xp @ Wvo (+cvo) ----
            cvo_b = None
            if not t_cvo:
                cvo_b = vbc.tile([128, H], F32, tag="vbc")
                nc.sync.dma_start(out=cvo_b[:], in_=bcast_row(1))
            VO = big.tile([128, TT, H], F32, tag="bufV")
            wvo_sb = []
            for nh in range(2):
                wt = wp.tile([128, KT, 512], F32R, tag="wset")
                nc.sync.dma_start(out=wt[:],
                                  in_=wvo_d[:, :, nh * 512:(nh + 1) * 512].rearrange("k p n -> p k n"))
                wvo_sb.append(wt)
            for grp in (range(g, g + 2) for g in range(0, TT, 2)):
                for nh in range(2):
                    sl = slice(nh * 512, (nh + 1) * 512)
                    for t in grp:
                        ps = psmm.tile([128, 512], F32, tag="mm")
                        for k in range(KT):
                            nc.tensor.matmul(ps[:], xpT[:, k, t, :], wvo_sb[nh][:, k, :],
                                             start=(k == 0), stop=(k == KT - 1))
                        if t_cvo:
                            nc.scalar.copy(out=VO[:, t, sl], in_=ps[:])
                        else:
                            nc.vector.tensor_add(VO[:, t, sl], ps[:], cvo_b[:, sl])

            # ---- blend + residual + LN1 + x1T transposes (per tile) ----
            if not t_ln1:
                g1_b = vbc.tile([128, H], F32, tag="vbc")
                nc.sync.dma_start(out=g1_b[:], in_=bcast_row(2))
                be1_b = vbc.tile([128, H], F32, tag="vbc")
                nc.sync.dma_start(out=be1_b[:], in_=bcast_row(3))
            x1T = big.tile([128, KT, TT, 128], F32R, tag="bufXT")
            for t in range(TT):
                vs = shufp.tile([128, H], F32, tag="shuf")
                nc.vector.stream_shuffle(vs[:], VO[:, t, :], SWAP_MASK)
                # r1 = xp + w*VO + (1-w)*VO_swap via two fused scalar_tensor_tensor
                tmp = dxp.tile([128, H], F32, tag="dxt")
                nc.vector.scalar_tensor_tensor(out=tmp[:], in0=vs[:],
                                               scalar=sigw1m[:, t:t + 1],
                                               in1=xp[:, t, :], op0=OP.mult, op1=OP.add)
                nc.vector.scalar_tensor_tensor(out=xp[:, t, :], in0=VO[:, t, :],
                                               scalar=sigw[:, t:t + 1],
                                               in1=tmp[:], op0=OP.mult, op1=OP.add)

                # LN1 -> x1 into VO slice
                sb = stat.tile([128, 4, 6], F32, tag="stat")
                mu, rstd = sb[:, 2, 2:3], sb[:, 2, 3:4]
                nc.vector.bn_stats(sb[:, 0, :], xp[:, t, 0:512])
                nc.vector.bn_stats(sb[:, 1, :], xp[:, t, 512:1024])
                nc.vector.bn_aggr(sb[:, 3, 0:2], sb[:, 0:2, :])
                nc.vector.tensor_copy(mu, sb[:, 3, 0:1])
                nc.scalar.activation(rstd, sb[:, 3, 1:2], AF.Sqrt, bias=epst[:], scale=1.0)
                nc.vector.reciprocal(rstd, rstd)
                nc.vector.tensor_scalar(out=VO[:, t, :], in0=xp[:, t, :],
                                        scalar1=mu, scalar2=rstd,
                                        op0=OP.subtract, op1=OP.mult)
                if not t_ln1:
                    nc.vector.tensor_mul(VO[:, t, :], VO[:, t, :], g1_b[:])
                    nc.vector.tensor_add(VO[:, t, :], VO[:, t, :], be1_b[:])
                for k in range(KT):
                    pst = pstp.tile([128, 128], F32, tag="tp")
                    nc.tensor.transpose(pst[:], VO[:, t, k * 128:(k + 1) * 128], ident[:])
                    nc.scalar.copy(out=x1T[:, k, t, :], in_=pst[:])

            # ---- y2acc seed: VO already holds x1; add b2 only if nontrivial ----
            if not t_b2:
                b2_b = vbc.tile([128, H], F32, tag="vbc")
                nc.sync.dma_start(out=b2_b[:], in_=bcast_row(6))
                for t in range(TT):
                    nc.vector.tensor_add(VO[:, t, :], VO[:, t, :], b2_b[:])

            if not t_ln2:
                g2_b = vbc.tile([128, H], F32, tag="vbc")
                nc.sync.dma_start(out=g2_b[:], in_=bcast_row(4))
                be2_b = vbc.tile([128, H], F32, tag="vbc")
                nc.sync.dma_start(out=be2_b[:], in_=bcast_row(5))

            # ---- FFN, feature-halved; y2 accumulates into VO (residual seeded) ----
            h1a = big.tile([128, KT, T], F32R, tag="bufX")
            h1b = big.tile([128, KT, T], F32R, tag="bufT")
            for fh in range(2):
                th_passes = [(0, 1), (2, 3)] if fh == 0 else [(0, 1, 2, 3)]
                for ths in th_passes:
                    for ml in range(16):
                        m = fh * 16 + ml
                        w1t = w1p.tile([128, KT, 128], F32R, tag="w1t")
                        nc.sync.dma_start(out=w1t[:],
                                          in_=w1_d[:, :, m * 128:(m + 1) * 128].rearrange("k p f -> p k f"))
                        dst = h1a if ml < 8 else h1b
                        mi = ml % 8
                        for th in ths:
                            ps2 = psmm2.tile([128, 512], F32, tag="mm2")
                            ps = ps2[:, 0:256]
                            for k in range(KT):
                                nc.tensor.matmul(ps[:], w1t[:, k, :],
                                                 x1T[:, k, th * 2:(th + 1) * 2, :],
                                                 start=(k == 0), stop=(k == KT - 1))
                            nc.scalar.activation(out=dst[:, mi, th * 256:(th + 1) * 256],
                                                 in_=ps[:], func=AF.Gelu,
                                                 bias=b1t[:, m:m + 1], scale=1.0)
                def ffn2_mm(kg, nh, t, ps):
                    for k8 in range(8):
                        kk = kg * 8 + k8
                        src = h1a if kk < 8 else h1b
                        nc.tensor.matmul(ps[:], src[:, kk % 8, t * 128:(t + 1) * 128],
                                         w2ts[(kg, nh)][:, k8, :],
                                         start=(k8 == 0), stop=(k8 == 7))

                w2ts = {}

                def load_w2(kg, nh):
                    sl = slice(nh * 512, (nh + 1) * 512)
                    w2t = wp.tile([128, 8, 512], F32R, tag="wset")
                    lo = fh * 16 + kg * 8
                    nc.sync.dma_start(out=w2t[:],
                                      in_=w2_d[lo:lo + 8, :, sl].rearrange("k p n -> p k n"))
                    w2ts[(kg, nh)] = w2t

                if fh == 0:
                    groups = [(0, 0), (0, 1), (1, 0), (1, 1)]
                else:
                    groups = [(0, 0), (0, 1)]
                for kg, nh in groups:
                    load_w2(kg, nh)
                    sl = slice(nh * 512, (nh + 1) * 512)
                    for t in range(TT):
                        ps = psmm.tile([128, 512], F32, tag="mm")
                        ffn2_mm(kg, nh, t, ps)
                        nc.vector.tensor_add(VO[:, t, sl], VO[:, t, sl], ps[:])
                if fh == 1:
                    # final contraction group: per-token nh0+nh1 + inline LN2+store
                    load_w2(1, 0)
                    load_w2(1, 1)
                    for t in range(TT):
                        for nh in range(2):
                            sl = slice(nh * 512, (nh + 1) * 512)
                            if nh == 1:
                                ps = psmm2.tile([128, 512], F32, tag="mm2")
                            else:
                                ps = psmm.tile([128, 512], F32, tag="mm")
                            ffn2_mm(1, nh, t, ps)
                            nc.vector.tensor_add(VO[:, t, sl], VO[:, t, sl], ps[:])
                        sb = stat.tile([128, 4, 6], F32, tag="stat")
                        mu, rstd = sb[:, 2, 2:3], sb[:, 2, 3:4]
                        nc.vector.bn_stats(sb[:, 0, :], VO[:, t, 0:512])
                        nc.vector.bn_stats(sb[:, 1, :], VO[:, t, 512:1024])
                        nc.vector.bn_aggr(sb[:, 3, 0:2], sb[:, 0:2, :])
                        nc.vector.tensor_copy(mu, sb[:, 3, 0:1])
                        nc.scalar.activation(rstd, sb[:, 3, 1:2], AF.Sqrt,
                                             bias=epst[:], scale=1.0)
                        nc.vector.reciprocal(rstd, rstd)
                        ot = dxp.tile([128, H], F32, tag="dxt")
                        nc.vector.tensor_scalar(out=ot[:], in0=VO[:, t, :],
                                                scalar1=mu, scalar2=rstd,
                                                op0=OP.subtract, op1=OP.mult)
                        if not t_ln2:
                            nc.vector.tensor_mul(ot[:], ot[:], g2_b[:])
                            nc.vector.tensor_add(ot[:], ot[:], be2_b[:])
                        nc.sync.dma_start(out=out_d[t, :, :], in_=ot[:])

    nc.compile()
    return nc


_NC_CACHE = {}


def _get_nc(triv):
    if triv not in _NC_CACHE:
        _NC_CACHE[triv] = build_nc(triv)
    return _NC_CACHE[triv]


def _host_prep(x, Wq, bq, Wk, bk, Wv, bv, Wo, bo, g1, be1, g2, be2, W1, b1, W2, b2, perm):
    f = lambda a: np.ascontiguousarray(np.asarray(a, dtype=np.float32))
    x, Wq, bq, Wk, bk, Wv, bv, Wo, bo = map(f, (x, Wq, bq, Wk, bk, Wv, bv, Wo, bo))
    g1, be1, g2, be2, W1, b1, W2, b2 = map(f, (g1, be1, g2, be2, W1, b1, W2, b2))
    perm = np.asarray(perm, dtype=np.int64)

    Wqk = np.ascontiguousarray(Wq @ Wk.T)
    ck = Wk @ bq
    Wvo = np.ascontiguousarray(Wv @ Wo)
    cvo = bv @ Wo + bo
    vecs = np.ascontiguousarray(np.stack([ck, cvo, g1, be1, g2, be2, b2]))
    b1t = np.ascontiguousarray(b1.reshape(MT, 128).T)

    triv = (
        bool(np.all(ck == 0.0)),
        bool(np.all(cvo == 0.0)),
        bool(np.all(g1 == 1.0) and np.all(be1 == 0.0)),
        bool(np.all(b2 == 0.0)),
        bool(np.all(g2 == 1.0) and np.all(be2 == 0.0)),
    )

    xp = np.ascontiguousarray(x[:, perm, :].reshape(B * S, H))
    shards = xp.reshape(N_CORES, TT, 128, H)

    common = {
        "wqk": Wqk.reshape(KT, 128, H),
        "wvo": Wvo.reshape(KT, 128, H),
        "w1": W1.reshape(KT, 128, F1),
        "w2": W2.reshape(MT, 128, H),
        "b1t": b1t,
        "vecs": vecs,
    }
    in_maps = [dict(common, xp=np.ascontiguousarray(shards[c])) for c in range(N_CORES)]
    return in_maps, perm, triv


def kernel(**inputs):
    in_maps, perm, triv = _host_prep(**inputs)
    nc = _get_nc(triv)
    res = run_bass_kernel_spmd(nc, in_maps, core_ids=list(range(N_CORES)))
    op = np.concatenate([r["out"].reshape(T, H) for r in res.results], axis=0)
    op = op.reshape(B, S, H)
    out = np.empty_like(op)
    out[:, perm, :] = op
    return out
